# revision 13
# baseline (speedup 1.0000x reference)
"""Emformer block (pre-LN MHA + FFN, post-LN) on 8 Trainium2 NeuronCores.

Decomposition (zero replicated FLOPs, host reshard between phases):
  Phase 1 (token-sharded, 512 rows/core): LN0 (folded gammas) -> QKV
           projections in bf16. Outputs qT/kT (channel major) and v (token
           major) in bf16, staged in wide SBUF buffers and stored with one
           batched DMA each. The v bias is folded into the phase-3 residual
           input on the host (softmax rows sum to 1).
  Phase 2 (head-sharded, 4 (b,h) pairs/core): scores^T = k^T.T @ q^T with
           K=64 row packing, exp on ScalarE over 3-bank psum tiles, mask
           multiply on DVE in bf16 (2x mode), unnormalized attn^T via v
           augmented with a ones column (softmax denominator for free).
           attn^T accumulates in a [65, T] buffer, one store per (b, head).
  Phase 3 (token-sharded): denominator normalize + residual + LN1 -> FFN1
           (bf16, bias+relu fused on ScalarE) -> FFN2 (bf16, psum
           accumulation over the hidden dim, bias via ones-row matmul)
           -> residual -> LN2 with g2/b2.

DMA queues: a DMA occupies its issuing queue until it completes, so streams
are spread over SP / Activation-HWDGE / gpsimd-SWDGE such that no queue
blocks a dependent stream. All PE transposes use a bf16 identity
(1.0 cycles/row instead of 2.0 for f32).
"""

import ml_dtypes
import numpy as np

import concourse.bass as bass
import concourse.mybir as mybir
import concourse.tile as tile
from concourse import bacc
from concourse.bass_utils import run_bass_kernel_spmd
from concourse.masks import make_identity

F32 = mybir.dt.float32
F32R = mybir.dt.float32r
BF16 = mybir.dt.bfloat16
AF = mybir.ActivationFunctionType
OP = mybir.AluOpType

B, T, D, H, DH, FFN = 2, 2048, 1024, 16, 64, 4096
LN_EPS = 1e-3
NCORES = 8
NTOK = B * T              # 4096
TOK_PC = NTOK // NCORES   # 512 token rows per core (phases 1/3)
NT = TOK_PC // 128        # 4 token tiles per core
CB = D // 128             # 8 contraction blocks over D
FB = FFN // 128           # 32 blocks over FFN dim
NPAIR = (B * H) // NCORES # 4 (batch, head) pairs per core (phase 2)
MB = T // 128             # 16 key blocks
NBQ = T // 512            # 4 query blocks of 512

# phase-2 exp groups over the 16 key blocks: (start, size)
GROUPS = [(0, 3), (3, 3), (6, 3), (9, 3), (12, 2), (14, 2)]
MB2GRP = {}
for _gi, (_s, _n) in enumerate(GROUPS):
    for _j in range(_n):
        MB2GRP[_s + _j] = (_gi, _j)


def _ln_stats(nc, pool, xt, rows=128, d=D):
    """Return (mu, rstd) [rows,1] f32 tiles for layer norm over free dim."""
    nsub = d // 512
    stats = pool.tile([128, nsub, 6], F32, name="ln_stats", tag="ln_stats")
    xg = xt.rearrange("p (s q) -> p s q", s=nsub)
    for s in range(nsub):
        nc.vector.bn_stats(out=stats[:rows, s, :], in_=xg[:rows, s, :])
    mv = pool.tile([128, 2], F32, name="ln_mv", tag="ln_mv")
    nc.vector.bn_aggr(out=mv[:rows], in_=stats[:rows])
    eps_t = pool.tile([128, 1], F32, name="ln_eps", tag="ln_eps")
    nc.vector.memset(eps_t, LN_EPS)
    std = pool.tile([128, 1], F32, name="ln_std", tag="ln_std")
    nc.scalar.activation(out=std[:rows], in_=mv[:rows, 1:2], func=AF.Sqrt,
                         bias=eps_t[:rows], scale=1.0)
    rstd = pool.tile([128, 1], F32, name="ln_rstd", tag="ln_rstd")
    nc.vector.reciprocal(out=rstd[:rows], in_=std[:rows])
    return mv[:rows, 0:1], rstd


def build_phase1():
    nc = bacc.Bacc(None, target_bir_lowering=False)
    x_d = nc.dram_tensor("x", [TOK_PC, D], F32, kind="ExternalInput")
    wq_d = nc.dram_tensor("wq", [128, CB, D], BF16, kind="ExternalInput")
    wk_d = nc.dram_tensor("wk", [128, CB, D], BF16, kind="ExternalInput")
    wv_d = nc.dram_tensor("wv", [128, CB, D], BF16, kind="ExternalInput")
    bq_d = nc.dram_tensor("bq", [CB, 128], F32, kind="ExternalInput")
    bk_d = nc.dram_tensor("bk", [CB, 128], F32, kind="ExternalInput")
    qT_o = nc.dram_tensor("qT", [D, TOK_PC], BF16, kind="ExternalOutput")
    kT_o = nc.dram_tensor("kT", [D, TOK_PC], BF16, kind="ExternalOutput")
    v_o = nc.dram_tensor("v", [TOK_PC, D], BF16, kind="ExternalOutput")

    with tile.TileContext(nc) as tc:
        with (
            tc.tile_pool(name="const", bufs=1) as const,
            tc.tile_pool(name="w", bufs=1) as wpool,
            tc.tile_pool(name="xin", bufs=2) as xin,
            tc.tile_pool(name="small", bufs=4) as small,
            tc.tile_pool(name="ln", bufs=2) as lnp,
            tc.tile_pool(name="lnT", bufs=1) as lnTp,
            tc.tile_pool(name="obuf", bufs=1) as obuf,
            tc.tile_pool(name="pst", bufs=2, space="PSUM") as pst,
            tc.tile_pool(name="psq", bufs=4, space="PSUM") as psq,
        ):
            ident = const.tile([128, 128], BF16)
            make_identity(nc, ident)
            bq_sb = const.tile([128, CB], F32)
            nc.sync.dma_start(out=bq_sb, in_=bq_d[:, :].rearrange("c p -> p c"))
            bk_sb = const.tile([128, CB], F32)
            nc.sync.dma_start(out=bk_sb, in_=bk_d[:, :].rearrange("c p -> p c"))

            # weights on the Activation DMA queue, x stream on SP
            w_sb = {}
            for name, wd in (("q", wq_d), ("k", wk_d), ("v", wv_d)):
                t = wpool.tile([128, CB, D], BF16, name=f"w{name}",
                               tag=f"w{name}")
                nc.scalar.dma_start(out=t, in_=wd[:, :, :])
                w_sb[name] = t

            # LN0 -> ln_t (bf16) -> transpose -> lnT
            lnT = lnTp.tile([128, CB, TOK_PC], BF16, name="lnT", tag="lnT")
            for nt in range(NT):
                xt = xin.tile([128, D], F32, name="xt", tag="xt")
                nc.sync.dma_start(out=xt,
                                  in_=x_d[nt * 128:(nt + 1) * 128, :])
                mu, rstd = _ln_stats(nc, small, xt)
                ln_t = lnp.tile([128, D], BF16, name="ln_t", tag="ln_t")
                nc.gpsimd.tensor_scalar(out=ln_t, in0=xt, scalar1=mu,
                                        scalar2=rstd, op0=OP.subtract,
                                        op1=OP.mult)
                for cb in range(CB):
                    tp = pst.tile([128, 128], BF16, name="tp", tag="tp")
                    nc.tensor.transpose(
                        tp, ln_t[:, cb * 128:(cb + 1) * 128], ident)
                    nc.vector.tensor_copy(
                        out=lnT[:, cb, nt * 128:(nt + 1) * 128], in_=tp)

            # qT / kT: psum [128 dout, 512 tok] per d-block; results stage
            # in [128, CB, 512] buffers; one batched store each.
            for name, bias_sb, out_d, eng in (("q", bq_sb, qT_o, "v"),
                                              ("k", bk_sb, kT_o, "s")):
                w = w_sb[name]
                ob = obuf.tile([128, CB, TOK_PC], BF16, name=f"ob{name}",
                               tag=f"ob{name}")
                for db in range(CB):
                    ps = psq.tile([128, 512], F32, name="psqk", tag="psqv")
                    for cb in range(CB):
                        nc.tensor.matmul(
                            ps, w[:, cb, db * 128:(db + 1) * 128],
                            lnT[:, cb, :],
                            start=(cb == 0), stop=(cb == CB - 1))
                    if eng == "v":
                        nc.vector.tensor_scalar(
                            out=ob[:, db, :], in0=ps,
                            scalar1=bias_sb[:, db:db + 1], scalar2=None,
                            op0=OP.add)
                    else:
                        nc.scalar.activation(
                            out=ob[:, db, :], in_=ps, func=AF.Identity,
                            bias=bias_sb[:, db:db + 1], scale=1.0)
                nc.sync.dma_start(
                    out=out_d[:, :].rearrange("(db p) t -> p db t", p=128),
                    in_=ob)

            # v: psum [128 tok, 512 dout]; bias folded into phase-3 x
            wv = w_sb["v"]
            vb = obuf.tile([128, NT, D], BF16, name="vb", tag="vb")
            for nt in range(NT):
                for dh in range(2):
                    ps = psq.tile([128, 512], F32, name="psv", tag="psqv")
                    for cb in range(CB):
                        nc.tensor.matmul(
                            ps, lnT[:, cb, nt * 128:(nt + 1) * 128],
                            wv[:, cb, dh * 512:(dh + 1) * 512],
                            start=(cb == 0), stop=(cb == CB - 1))
                    nc.scalar.activation(
                        out=vb[:, nt, dh * 512:(dh + 1) * 512], in_=ps,
                        func=AF.Identity, scale=1.0)
            nc.sync.dma_start(
                out=v_o[:, :].rearrange("(nt p) d -> p nt d", p=128),
                in_=vb)

    nc.compile()
    return nc


def build_phase2():
    nc = bacc.Bacc(None, target_bir_lowering=False)
    qT_d = nc.dram_tensor("qT", [NPAIR, DH, T], BF16, kind="ExternalInput")
    kT_d = nc.dram_tensor("kT", [NPAIR, DH, T], BF16, kind="ExternalInput")
    v_d = nc.dram_tensor("v", [NPAIR, 128, MB, DH + 1], BF16,
                         kind="ExternalInput")
    mT_d = nc.dram_tensor("maskT", [B, 128, MB, T], BF16,
                          kind="ExternalInput")
    attn_o = nc.dram_tensor("attn", [NPAIR, DH + 1, T], BF16,
                            kind="ExternalOutput")

    with tile.TileContext(nc) as tc:
        with (
            tc.tile_pool(name="mask", bufs=3) as maskp,
            tc.tile_pool(name="qk", bufs=2) as qkp,
            tc.tile_pool(name="vp", bufs=2) as vp,
            tc.tile_pool(name="em", bufs=2) as emp,
            tc.tile_pool(name="at", bufs=2) as atp,
            tc.tile_pool(name="pss", bufs=2, space="PSUM") as pss,
            tc.tile_pool(name="psa", bufs=2, space="PSUM") as psa,
        ):
            for b in range(B):
                qs, ks, vs, abufs = [], [], [], []
                for hp in range(2):
                    p = b * 2 + hp
                    qsb = qkp.tile([128, T], BF16, name=f"qsb{hp}",
                                   tag=f"q{hp}")
                    ksb = qkp.tile([128, T], BF16, name=f"ksb{hp}",
                                   tag=f"k{hp}")
                    nc.gpsimd.dma_start(out=qsb[0:DH, :], in_=qT_d[p])
                    nc.gpsimd.dma_start(out=qsb[DH:128, :], in_=qT_d[p])
                    nc.gpsimd.dma_start(out=ksb[0:DH, :], in_=kT_d[p])
                    nc.gpsimd.dma_start(out=ksb[DH:128, :], in_=kT_d[p])
                    vsb = vp.tile([128, MB, DH + 1], BF16, name=f"vsb{hp}",
                                  tag=f"v{hp}")
                    nc.gpsimd.dma_start(out=vsb, in_=v_d[p])
                    ab = atp.tile([DH + 1, T], BF16, name=f"ab{hp}",
                                  tag=f"ab{hp}")
                    qs.append(qsb); ks.append(ksb); vs.append(vsb)
                    abufs.append(ab)
                for nb in range(NBQ):
                    ns = slice(nb * 512, (nb + 1) * 512)
                    mt = maskp.tile([128, MB, 512], BF16, name="mt",
                                    tag="mask")
                    nc.gpsimd.dma_start(out=mt, in_=mT_d[b][:, :, ns])
                    for hp in range(2):
                        qsb, ksb, vsb = qs[hp], ks[hp], vs[hp]
                        em = []
                        for gi, (s0, gn) in enumerate(GROUPS):
                            ps = pss.tile([128, 3, 512], F32, name="ps_s",
                                          tag="scores")
                            for j in range(gn):
                                mb = s0 + j
                                lo = (mb % 2) * DH
                                nc.tensor.matmul(
                                    ps[:, j, :],
                                    ksb[lo:lo + DH,
                                        mb * 128:(mb + 1) * 128],
                                    qsb[lo:lo + DH, ns],
                                    start=True, stop=True,
                                    tile_position=(lo, 0))
                            emv = emp.tile([128, 3, 512], BF16,
                                           name=f"em{gi}", tag=f"em{gi}")
                            nc.scalar.activation(
                                out=emv[:, :gn, :], in_=ps[:, :gn, :],
                                func=AF.Exp, scale=0.125)
                            nc.vector.tensor_tensor(
                                out=emv[:, :gn, :], in0=emv[:, :gn, :],
                                in1=mt[:, s0:s0 + gn, :], op=OP.mult)
                            em.append(emv)
                        pa = psa.tile([DH + 1, 512], F32, name="pa",
                                      tag="attn")
                        for mb in range(MB):
                            gi, j = MB2GRP[mb]
                            nc.tensor.matmul(pa, vsb[:, mb, :],
                                             em[gi][:, j, :],
                                             start=(mb == 0),
                                             stop=(mb == MB - 1))
                        nc.vector.tensor_copy(out=abufs[hp][:, ns], in_=pa)
                for hp in range(2):
                    nc.sync.dma_start(out=attn_o[b * 2 + hp],
                                      in_=abufs[hp])

    nc.compile()
    return nc


def build_phase3():
    nc = bacc.Bacc(None, target_bir_lowering=False)
    attn_d = nc.dram_tensor("attn", [TOK_PC, D], BF16, kind="ExternalInput")
    den_d = nc.dram_tensor("den", [TOK_PC, H], F32, kind="ExternalInput")
    x_d = nc.dram_tensor("x", [TOK_PC, D], F32, kind="ExternalInput")
    # w1 chunked [chunk, 128, 4 fb, CB, 128] bf16; w2 halves [128, FB, 512]
    wf1_d = nc.dram_tensor("wf1", [8, 128, 4, CB, 128], BF16,
                           kind="ExternalInput")
    bf1_d = nc.dram_tensor("bf1", [FB, 128], F32, kind="ExternalInput")
    wf2a_d = nc.dram_tensor("wf2a", [128, FB, 512], BF16,
                            kind="ExternalInput")
    wf2b_d = nc.dram_tensor("wf2b", [128, FB, 512], BF16,
                            kind="ExternalInput")
    bf2_d = nc.dram_tensor("bf2", [1, D], BF16, kind="ExternalInput")
    onesb_d = nc.dram_tensor("onesb", [1, 128], BF16, kind="ExternalInput")
    g2_d = nc.dram_tensor("g2", [1, D], F32, kind="ExternalInput")
    b2_d = nc.dram_tensor("b2", [1, D], F32, kind="ExternalInput")
    out_o = nc.dram_tensor("out", [TOK_PC, D], F32, kind="ExternalOutput")

    with tile.TileContext(nc) as tc:
        with (
            tc.tile_pool(name="const", bufs=1) as const,
            tc.tile_pool(name="xin", bufs=2) as xin,
            tc.tile_pool(name="small", bufs=4) as small,
            tc.tile_pool(name="ao", bufs=1) as aop,
            tc.tile_pool(name="ln", bufs=2) as lnp,
            tc.tile_pool(name="lnT", bufs=1) as lnTp,
            tc.tile_pool(name="w1", bufs=3) as w1p,
            tc.tile_pool(name="w2", bufs=2) as w2p,
            tc.tile_pool(name="w2r", bufs=1) as w2rp,
            tc.tile_pool(name="t1", bufs=1) as t1p,
            tc.tile_pool(name="y", bufs=1) as yp,
            tc.tile_pool(name="outp", bufs=2) as outp,
            tc.tile_pool(name="pst", bufs=2, space="PSUM") as pst,
            tc.tile_pool(name="psf", bufs=2, space="PSUM") as psf,
            tc.tile_pool(name="psy", bufs=1, space="PSUM") as psyp,
        ):
            ident = const.tile([128, 128], BF16)
            make_identity(nc, ident)
            onesb_t = const.tile([1, 128], BF16)
            nc.sync.dma_start(out=onesb_t, in_=onesb_d[:, :])
            bf1_sb = const.tile([128, FB], F32)
            nc.sync.dma_start(out=bf1_sb,
                              in_=bf1_d[:, :].rearrange("f p -> p f"))
            bf2_sb = const.tile([1, D], BF16)
            nc.sync.dma_start(out=bf2_sb, in_=bf2_d[:, :])
            g2_sb = const.tile([128, D], F32)
            nc.sync.dma_start(out=g2_sb, in_=bass.AP(
                tensor=g2_d, offset=0, ap=[[0, 128], [1, D]]))
            b2_sb = const.tile([128, D], F32)
            nc.sync.dma_start(out=b2_sb, in_=bass.AP(
                tensor=b2_d, offset=0, ap=[[0, 128], [1, D]]))
            # resident second-half FFN2 weights on the gpsimd queue
            w2r = w2rp.tile([128, FB, 512], BF16, name="w2r", tag="w2r")
            nc.gpsimd.dma_start(out=w2r, in_=wf2b_d[:, :, :])

            # residual 1 + LN1 + transpose
            lnT = lnTp.tile([128, CB, TOK_PC], BF16, name="lnT", tag="lnT")
            ao_t = []
            for nt in range(NT):
                at = xin.tile([128, D], BF16, name="at", tag="attn_in")
                nc.sync.dma_start(out=at,
                                  in_=attn_d[nt * 128:(nt + 1) * 128, :])
                xt = xin.tile([128, D], F32, name="xt", tag="x_in")
                nc.sync.dma_start(out=xt,
                                  in_=x_d[nt * 128:(nt + 1) * 128, :])
                den = small.tile([128, H], F32, name="den", tag="den")
                nc.sync.dma_start(out=den,
                                  in_=den_d[nt * 128:(nt + 1) * 128, :])
                rec = small.tile([128, H], F32, name="recd", tag="recd")
                nc.vector.reciprocal(out=rec, in_=den)
                an = lnp.tile([128, H, DH], F32, name="an", tag="an",
                              bufs=1)
                rec_bc = bass.AP(tensor=rec.tensor, offset=rec.offset,
                                 ap=[rec.ap[0], rec.ap[1], [0, DH]])
                nc.vector.tensor_tensor(
                    out=an, in0=at.rearrange("p (h d) -> p h d", h=H),
                    in1=rec_bc, op=OP.mult)
                ao = aop.tile([128, D], F32, name=f"ao{nt}", tag=f"ao{nt}")
                nc.gpsimd.tensor_tensor(
                    out=ao.rearrange("p (h d) -> p h d", h=H),
                    in0=an,
                    in1=xt.rearrange("p (h d) -> p h d", h=H), op=OP.add)
                ao_t.append(ao)
                mu, rstd = _ln_stats(nc, small, ao)
                ln_t = lnp.tile([128, D], BF16, name="ln3", tag="ln3")
                nc.vector.tensor_scalar(out=ln_t, in0=ao, scalar1=mu,
                                        scalar2=rstd, op0=OP.subtract,
                                        op1=OP.mult)
                for cb in range(CB):
                    tp = pst.tile([128, 128], BF16, name="tp", tag="tp")
                    nc.tensor.transpose(
                        tp, ln_t[:, cb * 128:(cb + 1) * 128], ident)
                    nc.vector.tensor_copy(
                        out=lnT[:, cb, nt * 128:(nt + 1) * 128], in_=tp)

            # FFN1: t1[f-part, fb, n], bf16 weights in 8 chunked DMAs,
            # bias+relu fused on ScalarE
            t1 = [t1p.tile([128, TOK_PC], BF16, name=f"t1_{fb}",
                           tag=f"t1_{fb}") for fb in range(FB)]
            for ch in range(8):
                w1 = w1p.tile([128, 4, CB, 128], BF16, name="w1", tag="w1")
                nc.sync.dma_start(out=w1, in_=wf1_d[ch])
                for f4 in range(4):
                    fb = ch * 4 + f4
                    ps = psf.tile([128, TOK_PC], F32, name="psf", tag="psf")
                    for cb in range(CB):
                        nc.tensor.matmul(ps, w1[:, f4, cb, :],
                                         lnT[:, cb, :],
                                         start=(cb == 0),
                                         stop=(cb == CB - 1))
                    nc.scalar.activation(out=t1[fb], in_=ps,
                                         func=AF.Relu,
                                         bias=bf1_sb[:, fb:fb + 1],
                                         scale=1.0)

            # FFN2 accumulating in psum; two passes over d halves
            y_t = [yp.tile([128, D], F32, name=f"y{nt}", tag=f"y{nt}")
                   for nt in range(NT)]
            # d-half 0: fb-outer with streamed weights (8 chunked DMAs)
            dsl0 = slice(0, 512)
            pys = [psyp.tile([128, 512], F32, name=f"psy{nt}",
                             tag=f"psy{nt}") for nt in range(NT)]
            for ch in range(8):
                w2 = w2p.tile([128, 4, 512], BF16, name="w2", tag="w2")
                nc.gpsimd.dma_start(out=w2,
                                    in_=wf2a_d[:, ch * 4:(ch + 1) * 4, :])
                for f4 in range(4):
                    fb = ch * 4 + f4
                    for nt in range(NT):
                        nc.tensor.matmul(
                            pys[nt], t1[fb][:, nt * 128:(nt + 1) * 128],
                            w2[:, f4, :], start=(fb == 0), stop=False)
            for nt in range(NT):
                nc.tensor.matmul(pys[nt], onesb_t, bf2_sb[0:1, dsl0],
                                 start=False, stop=True)
                nc.vector.tensor_tensor(out=y_t[nt][:, dsl0],
                                        in0=pys[nt],
                                        in1=ao_t[nt][:, dsl0],
                                        op=OP.add)
            # d-half 1: resident weights, nt-major so each token tile's
            # LN2 overlaps the remaining matmuls
            dsl1 = slice(512, 1024)
            for nt in range(NT):
                py = psyp.tile([128, 512], F32, name=f"psyb{nt}",
                               tag=f"psy{nt}")
                for fb in range(FB):
                    nc.tensor.matmul(
                        py, t1[fb][:, nt * 128:(nt + 1) * 128],
                        w2r[:, fb, :], start=(fb == 0), stop=False)
                nc.tensor.matmul(py, onesb_t, bf2_sb[0:1, dsl1],
                                 start=False, stop=True)
                nc.vector.tensor_tensor(out=y_t[nt][:, dsl1],
                                        in0=py,
                                        in1=ao_t[nt][:, dsl1],
                                        op=OP.add)
                mu, rstd = _ln_stats(nc, small, y_t[nt])
                z = lnp.tile([128, D], F32, name="z", tag="z")
                nc.vector.tensor_scalar(out=z, in0=y_t[nt], scalar1=mu,
                                        scalar2=rstd, op0=OP.subtract,
                                        op1=OP.mult)
                nc.gpsimd.tensor_tensor(out=z, in0=z, in1=g2_sb,
                                        op=OP.mult)
                ot = outp.tile([128, D], F32, name="ot", tag="out")
                nc.vector.tensor_tensor(out=ot, in0=z, in1=b2_sb,
                                        op=OP.add)
                nc.scalar.dma_start(out=out_o[nt * 128:(nt + 1) * 128, :],
                                    in_=ot)

    nc.compile()
    return nc


_CACHE = {}


def _get(name, builder):
    if name not in _CACHE:
        _CACHE[name] = builder()
    return _CACHE[name]


def kernel(x, mask, Wq, bq, Wk, bk, Wv, bv, g_in, b_in, g1, b1,
           W_ff1, b_ff1, W_ff2, b_ff2, g2, b2):
    f = np.float32
    bf = ml_dtypes.bfloat16
    x = np.asarray(x, f)
    xf = x.reshape(NTOK, D)

    def wprep(W):
        Wf = np.asarray(g_in, f)[:, None] * np.asarray(W, f)
        return np.ascontiguousarray(
            Wf.reshape(CB, 128, D).transpose(1, 0, 2)).astype(bf)

    Wq_b, Wk_b, Wv_b = wprep(Wq), wprep(Wk), wprep(Wv)
    bq_f = (b_in @ Wq + bq).astype(f).reshape(CB, 128)
    bk_f = (b_in @ Wk + bk).astype(f).reshape(CB, 128)
    bv_f = np.asarray(b_in @ Wv + bv, f)
    Wf1_f = np.ascontiguousarray(
        (g1[:, None] * W_ff1).astype(f).reshape(CB, 128, 8, 4, 128)
        .transpose(2, 1, 3, 0, 4)).astype(bf)
    bf1_f = (b1 @ W_ff1 + b_ff1).astype(f).reshape(FB, 128)
    Wf2 = np.asarray(W_ff2, f)
    Wf2a = np.ascontiguousarray(
        Wf2[:, 0:512].reshape(FB, 128, 512).transpose(1, 0, 2)).astype(bf)
    Wf2b = np.ascontiguousarray(
        Wf2[:, 512:1024].reshape(FB, 128, 512).transpose(1, 0, 2)).astype(bf)
    bf2_f = np.asarray(b_ff2, bf).reshape(1, D)
    x3 = xf + bv_f[None, :]
    maskT = np.ascontiguousarray(
        np.asarray(mask)[:, 0].transpose(0, 2, 1)
        .reshape(B, MB, 128, T).transpose(0, 2, 1, 3)).astype(bf)

    cores = list(range(NCORES))

    # ---- phase 1
    nc1 = _get("p1", build_phase1)
    in1 = [{
        "x": xf[c * TOK_PC:(c + 1) * TOK_PC],
        "wq": Wq_b, "wk": Wk_b, "wv": Wv_b,
        "bq": bq_f, "bk": bk_f,
    } for c in cores]
    r1 = run_bass_kernel_spmd(nc1, in1, cores)
    qT = np.concatenate([r1.results[c]["qT"] for c in cores], axis=1)
    kT = np.concatenate([r1.results[c]["kT"] for c in cores], axis=1)
    v = np.concatenate([r1.results[c]["v"] for c in cores], axis=0)

    # ---- phase 2
    nc2 = _get("p2", build_phase2)
    onecol = np.ones((T, 1), bf)
    in2 = []
    for c in cores:
        qs, ks, vs = [], [], []
        for b in range(B):
            for hp in range(2):
                h = 2 * c + hp
                qs.append(qT[h * DH:(h + 1) * DH, b * T:(b + 1) * T])
                ks.append(kT[h * DH:(h + 1) * DH, b * T:(b + 1) * T])
                va = np.concatenate(
                    [v[b * T:(b + 1) * T, h * DH:(h + 1) * DH], onecol],
                    axis=1)
                vs.append(np.ascontiguousarray(
                    va.reshape(MB, 128, DH + 1).transpose(1, 0, 2)))
        in2.append({
            "qT": np.ascontiguousarray(np.stack(qs)),
            "kT": np.ascontiguousarray(np.stack(ks)),
            "v": np.ascontiguousarray(np.stack(vs)),
            "maskT": maskT,
        })
    r2 = run_bass_kernel_spmd(nc2, in2, cores)
    attn = np.empty((NTOK, D), bf)
    den = np.empty((NTOK, H), f)
    for c in cores:
        i = 0
        for b in range(B):
            for hp in range(2):
                h = 2 * c + hp
                a65 = r2.results[c]["attn"][i]
                attn[b * T:(b + 1) * T, h * DH:(h + 1) * DH] = a65[0:DH, :].T
                den[b * T:(b + 1) * T, h] = a65[DH, :].astype(f)
                i += 1

    # ---- phase 3
    nc3 = _get("p3", build_phase3)
    in3 = [{
        "attn": attn[c * TOK_PC:(c + 1) * TOK_PC],
        "den": den[c * TOK_PC:(c + 1) * TOK_PC],
        "x": x3[c * TOK_PC:(c + 1) * TOK_PC],
        "wf1": Wf1_f, "bf1": bf1_f, "wf2a": Wf2a, "wf2b": Wf2b,
        "bf2": bf2_f,
        "onesb": np.ones((1, 128), bf),
        "g2": np.asarray(g2, f).reshape(1, D),
        "b2": np.asarray(b2, f).reshape(1, D),
    } for c in cores]
    r3 = run_bass_kernel_spmd(nc3, in3, cores)
    out = np.concatenate([r3.results[c]["out"] for c in cores], axis=0)
    return out.reshape(B, T, D)


# revision 14
# speedup vs baseline: 1.0541x; 1.0541x over previous
"""Emformer block (pre-LN MHA + FFN, post-LN) on 8 Trainium2 NeuronCores.

Decomposition (zero replicated FLOPs, host reshard between phases):
  Phase 1 (token-sharded, 512 rows/core): LN0 (folded gammas) -> QKV
           projections in bf16. Outputs qT/kT (channel major) and v (token
           major) in bf16, staged in wide SBUF buffers and stored with one
           batched DMA each. The v bias is folded into the phase-3 residual
           input on the host (softmax rows sum to 1).
  Phase 2 (head-sharded, 4 (b,h) pairs/core): scores^T = k^T.T @ q^T with
           K=64 row packing, exp on ScalarE over 3-bank psum tiles, mask
           multiply on DVE in bf16 (2x mode), unnormalized attn^T via v
           augmented with a ones column (softmax denominator for free).
           attn^T accumulates in a [65, T] buffer, one store per (b, head).
  Phase 3 (token-sharded): denominator normalize + residual + LN1 -> FFN1
           (bf16, bias+relu fused on ScalarE) -> FFN2 (bf16, psum
           accumulation over the hidden dim, bias via ones-row matmul)
           -> residual -> LN2 with g2/b2.

DMA queues: a DMA occupies its issuing queue until it completes, so streams
are spread over SP / Activation-HWDGE / gpsimd-SWDGE such that no queue
blocks a dependent stream. All PE transposes use a bf16 identity
(1.0 cycles/row instead of 2.0 for f32).
"""

import ml_dtypes
import numpy as np

import concourse.bass as bass
import concourse.mybir as mybir
import concourse.tile as tile
from concourse import bacc
from concourse.bass_utils import run_bass_kernel_spmd
from concourse.masks import make_identity

F32 = mybir.dt.float32
F32R = mybir.dt.float32r
BF16 = mybir.dt.bfloat16
AF = mybir.ActivationFunctionType
OP = mybir.AluOpType

B, T, D, H, DH, FFN = 2, 2048, 1024, 16, 64, 4096
LN_EPS = 1e-3
NCORES = 8
NTOK = B * T              # 4096
TOK_PC = NTOK // NCORES   # 512 token rows per core (phases 1/3)
NT = TOK_PC // 128        # 4 token tiles per core
CB = D // 128             # 8 contraction blocks over D
FB = FFN // 128           # 32 blocks over FFN dim
NPAIR = (B * H) // NCORES # 4 (batch, head) pairs per core (phase 2)
MB = T // 128             # 16 key blocks
NBQ = T // 512            # 4 query blocks of 512

# phase-2 exp groups over the 16 key blocks: (start, size)
GROUPS = [(0, 3), (3, 3), (6, 3), (9, 3), (12, 2), (14, 2)]
MB2GRP = {}
for _gi, (_s, _n) in enumerate(GROUPS):
    for _j in range(_n):
        MB2GRP[_s + _j] = (_gi, _j)


def _ln_stats(nc, pool, xt, rows=128, d=D):
    """Return (mu, rstd) [rows,1] f32 tiles for layer norm over free dim."""
    nsub = d // 512
    stats = pool.tile([128, nsub, 6], F32, name="ln_stats", tag="ln_stats")
    xg = xt.rearrange("p (s q) -> p s q", s=nsub)
    for s in range(nsub):
        nc.vector.bn_stats(out=stats[:rows, s, :], in_=xg[:rows, s, :])
    mv = pool.tile([128, 2], F32, name="ln_mv", tag="ln_mv")
    nc.vector.bn_aggr(out=mv[:rows], in_=stats[:rows])
    eps_t = pool.tile([128, 1], F32, name="ln_eps", tag="ln_eps")
    nc.vector.memset(eps_t, LN_EPS)
    std = pool.tile([128, 1], F32, name="ln_std", tag="ln_std")
    nc.scalar.activation(out=std[:rows], in_=mv[:rows, 1:2], func=AF.Sqrt,
                         bias=eps_t[:rows], scale=1.0)
    rstd = pool.tile([128, 1], F32, name="ln_rstd", tag="ln_rstd")
    nc.vector.reciprocal(out=rstd[:rows], in_=std[:rows])
    return mv[:rows, 0:1], rstd


def build_phase1():
    nc = bacc.Bacc(None, target_bir_lowering=False)
    x_d = nc.dram_tensor("x", [TOK_PC, D], BF16, kind="ExternalInput")
    wq_d = nc.dram_tensor("wq", [128, CB, D], BF16, kind="ExternalInput")
    wk_d = nc.dram_tensor("wk", [128, CB, D], BF16, kind="ExternalInput")
    wv_d = nc.dram_tensor("wv", [128, CB, D], BF16, kind="ExternalInput")
    bq_d = nc.dram_tensor("bq", [CB, 128], F32, kind="ExternalInput")
    bk_d = nc.dram_tensor("bk", [CB, 128], F32, kind="ExternalInput")
    qT_o = nc.dram_tensor("qT", [D, TOK_PC], BF16, kind="ExternalOutput")
    kT_o = nc.dram_tensor("kT", [D, TOK_PC], BF16, kind="ExternalOutput")
    v_o = nc.dram_tensor("v", [TOK_PC, D], BF16, kind="ExternalOutput")

    with tile.TileContext(nc) as tc:
        with (
            tc.tile_pool(name="const", bufs=1) as const,
            tc.tile_pool(name="w", bufs=1) as wpool,
            tc.tile_pool(name="xin", bufs=4) as xin,
            tc.tile_pool(name="small", bufs=4) as small,
            tc.tile_pool(name="ln", bufs=2) as lnp,
            tc.tile_pool(name="lnT", bufs=1) as lnTp,
            tc.tile_pool(name="obuf", bufs=1) as obuf,
            tc.tile_pool(name="pst", bufs=2, space="PSUM") as pst,
            tc.tile_pool(name="psq", bufs=4, space="PSUM") as psq,
        ):
            ident = const.tile([128, 128], BF16)
            make_identity(nc, ident)
            bq_sb = const.tile([128, CB], F32)
            nc.sync.dma_start(out=bq_sb, in_=bq_d[:, :].rearrange("c p -> p c"))
            bk_sb = const.tile([128, CB], F32)
            nc.sync.dma_start(out=bk_sb, in_=bk_d[:, :].rearrange("c p -> p c"))

            # x tiles first (critical path), then weights: the DMA engines
            # drain in arrival order, so issue order = priority
            x_ts = []
            for nt in range(NT):
                xt = xin.tile([128, D], BF16, name="xt", tag="xt")
                nc.sync.dma_start(out=xt,
                                  in_=x_d[nt * 128:(nt + 1) * 128, :])
                x_ts.append(xt)
            w_sb = {}
            for name, wd in (("q", wq_d), ("k", wk_d), ("v", wv_d)):
                t = wpool.tile([128, CB, D], BF16, name=f"w{name}",
                               tag=f"w{name}")
                nc.sync.dma_start(out=t, in_=wd[:, :, :])
                w_sb[name] = t

            # LN0 -> ln_t (bf16) -> transpose -> lnT
            lnT = lnTp.tile([128, CB, TOK_PC], BF16, name="lnT", tag="lnT")
            for nt in range(NT):
                xt = x_ts[nt]
                mu, rstd = _ln_stats(nc, small, xt)
                ln_t = lnp.tile([128, D], BF16, name="ln_t", tag="ln_t")
                nc.gpsimd.tensor_scalar(out=ln_t, in0=xt, scalar1=mu,
                                        scalar2=rstd, op0=OP.subtract,
                                        op1=OP.mult)
                for cb in range(CB):
                    tp = pst.tile([128, 128], BF16, name="tp", tag="tp")
                    nc.tensor.transpose(
                        tp, ln_t[:, cb * 128:(cb + 1) * 128], ident)
                    nc.vector.tensor_copy(
                        out=lnT[:, cb, nt * 128:(nt + 1) * 128], in_=tp)

            # qT / kT: psum [128 dout, 512 tok] per d-block; results stage
            # in [128, CB, 512] buffers; one batched store each.
            for name, bias_sb, out_d, eng in (("q", bq_sb, qT_o, "v"),
                                              ("k", bk_sb, kT_o, "s")):
                w = w_sb[name]
                ob = obuf.tile([128, CB, TOK_PC], BF16, name=f"ob{name}",
                               tag=f"ob{name}")
                for db in range(CB):
                    ps = psq.tile([128, 512], F32, name="psqk", tag="psqv")
                    for cb in range(CB):
                        nc.tensor.matmul(
                            ps, w[:, cb, db * 128:(db + 1) * 128],
                            lnT[:, cb, :],
                            start=(cb == 0), stop=(cb == CB - 1))
                    if eng == "v":
                        nc.vector.tensor_scalar(
                            out=ob[:, db, :], in0=ps,
                            scalar1=bias_sb[:, db:db + 1], scalar2=None,
                            op0=OP.add)
                    else:
                        nc.scalar.activation(
                            out=ob[:, db, :], in_=ps, func=AF.Identity,
                            bias=bias_sb[:, db:db + 1], scale=1.0)
                nc.gpsimd.dma_start(
                    out=out_d[:, :].rearrange("(db p) t -> p db t", p=128),
                    in_=ob)

            # v: psum [128 tok, 512 dout]; bias folded into phase-3 x
            wv = w_sb["v"]
            vb = obuf.tile([128, NT, D], BF16, name="vb", tag="vb")
            for nt in range(NT):
                for dh in range(2):
                    ps = psq.tile([128, 512], F32, name="psv", tag="psqv")
                    for cb in range(CB):
                        nc.tensor.matmul(
                            ps, lnT[:, cb, nt * 128:(nt + 1) * 128],
                            wv[:, cb, dh * 512:(dh + 1) * 512],
                            start=(cb == 0), stop=(cb == CB - 1))
                    nc.scalar.activation(
                        out=vb[:, nt, dh * 512:(dh + 1) * 512], in_=ps,
                        func=AF.Identity, scale=1.0)
            nc.gpsimd.dma_start(
                out=v_o[:, :].rearrange("(nt p) d -> p nt d", p=128),
                in_=vb)

    nc.compile()
    return nc


def build_phase2():
    nc = bacc.Bacc(None, target_bir_lowering=False)
    qT_d = nc.dram_tensor("qT", [NPAIR, DH, T], BF16, kind="ExternalInput")
    kT_d = nc.dram_tensor("kT", [NPAIR, DH, T], BF16, kind="ExternalInput")
    v_d = nc.dram_tensor("v", [NPAIR, 128, MB, DH + 1], BF16,
                         kind="ExternalInput")
    mT_d = nc.dram_tensor("maskT", [B, 128, MB, T], BF16,
                          kind="ExternalInput")
    attn_o = nc.dram_tensor("attn", [NPAIR, DH + 1, T], BF16,
                            kind="ExternalOutput")

    with tile.TileContext(nc) as tc:
        with (
            tc.tile_pool(name="mask", bufs=3) as maskp,
            tc.tile_pool(name="qk", bufs=2) as qkp,
            tc.tile_pool(name="vp", bufs=2) as vp,
            tc.tile_pool(name="em", bufs=2) as emp,
            tc.tile_pool(name="at", bufs=2) as atp,
            tc.tile_pool(name="pss", bufs=2, space="PSUM") as pss,
            tc.tile_pool(name="psa", bufs=2, space="PSUM") as psa,
        ):
            for b in range(B):
                qs, ks, vs, abufs = [], [], [], []
                for hp in range(2):
                    p = b * 2 + hp
                    qsb = qkp.tile([DH, T], BF16, name=f"qsb{hp}",
                                   tag=f"q{hp}")
                    ksb = qkp.tile([DH, T], BF16, name=f"ksb{hp}",
                                   tag=f"k{hp}")
                    nc.gpsimd.dma_start(out=qsb, in_=qT_d[p])
                    nc.gpsimd.dma_start(out=ksb, in_=kT_d[p])
                    vsb = vp.tile([128, MB, DH + 1], BF16, name=f"vsb{hp}",
                                  tag=f"v{hp}")
                    nc.gpsimd.dma_start(out=vsb, in_=v_d[p])
                    ab = atp.tile([DH + 1, T], BF16, name=f"ab{hp}",
                                  tag=f"ab{hp}")
                    qs.append(qsb); ks.append(ksb); vs.append(vsb)
                    abufs.append(ab)
                for nb in range(NBQ):
                    ns = slice(nb * 512, (nb + 1) * 512)
                    mt = maskp.tile([128, MB, 512], BF16, name="mt",
                                    tag="mask")
                    nc.gpsimd.dma_start(out=mt, in_=mT_d[b][:, :, ns])
                    for hp in range(2):
                        qsb, ksb, vsb = qs[hp], ks[hp], vs[hp]
                        em = []
                        for gi, (s0, gn) in enumerate(GROUPS):
                            ps = pss.tile([128, 3, 512], F32, name="ps_s",
                                          tag="scores")
                            for j in range(gn):
                                mb = s0 + j
                                nc.tensor.matmul(
                                    ps[:, j, :],
                                    ksb[:, mb * 128:(mb + 1) * 128],
                                    qsb[:, ns],
                                    start=True, stop=True)
                            emv = emp.tile([128, 3, 512], BF16,
                                           name=f"em{gi}", tag=f"em{gi}")
                            nc.scalar.activation(
                                out=emv[:, :gn, :], in_=ps[:, :gn, :],
                                func=AF.Exp, scale=0.125)
                            nc.vector.tensor_tensor(
                                out=emv[:, :gn, :], in0=emv[:, :gn, :],
                                in1=mt[:, s0:s0 + gn, :], op=OP.mult)
                            em.append(emv)
                        pa = psa.tile([DH + 1, 512], F32, name="pa",
                                      tag="attn")
                        for mb in range(MB):
                            gi, j = MB2GRP[mb]
                            nc.tensor.matmul(pa, vsb[:, mb, :],
                                             em[gi][:, j, :],
                                             start=(mb == 0),
                                             stop=(mb == MB - 1))
                        nc.vector.tensor_copy(out=abufs[hp][:, ns], in_=pa)
                for hp in range(2):
                    nc.sync.dma_start(out=attn_o[b * 2 + hp],
                                      in_=abufs[hp])

    nc.compile()
    return nc


def build_phase3():
    nc = bacc.Bacc(None, target_bir_lowering=False)
    attn_d = nc.dram_tensor("attn", [TOK_PC, D], BF16, kind="ExternalInput")
    den_d = nc.dram_tensor("den", [TOK_PC, H], F32, kind="ExternalInput")
    x_d = nc.dram_tensor("x", [TOK_PC, D], F32, kind="ExternalInput")
    # w1 chunked [chunk, 128, 4 fb, CB, 128] bf16; w2 halves [128, FB, 512]
    wf1_d = nc.dram_tensor("wf1", [8, 128, 4, CB, 128], BF16,
                           kind="ExternalInput")
    bf1_d = nc.dram_tensor("bf1", [FB, 128], F32, kind="ExternalInput")
    wf2a_d = nc.dram_tensor("wf2a", [128, FB, 512], BF16,
                            kind="ExternalInput")
    wf2b_d = nc.dram_tensor("wf2b", [128, FB, 512], BF16,
                            kind="ExternalInput")
    bf2_d = nc.dram_tensor("bf2", [1, D], BF16, kind="ExternalInput")
    onesb_d = nc.dram_tensor("onesb", [1, 128], BF16, kind="ExternalInput")
    g2_d = nc.dram_tensor("g2", [1, D], F32, kind="ExternalInput")
    b2_d = nc.dram_tensor("b2", [1, D], F32, kind="ExternalInput")
    out_o = nc.dram_tensor("out", [TOK_PC, D], F32, kind="ExternalOutput")

    with tile.TileContext(nc) as tc:
        with (
            tc.tile_pool(name="const", bufs=1) as const,
            tc.tile_pool(name="xin", bufs=4) as xin,
            tc.tile_pool(name="small", bufs=4) as small,
            tc.tile_pool(name="ao", bufs=1) as aop,
            tc.tile_pool(name="ln", bufs=2) as lnp,
            tc.tile_pool(name="lnT", bufs=1) as lnTp,
            tc.tile_pool(name="w1", bufs=3) as w1p,
            tc.tile_pool(name="w2", bufs=2) as w2p,
            tc.tile_pool(name="w2r", bufs=1) as w2rp,
            tc.tile_pool(name="t1", bufs=1) as t1p,
            tc.tile_pool(name="y", bufs=1) as yp,
            tc.tile_pool(name="outp", bufs=2) as outp,
            tc.tile_pool(name="pst", bufs=2, space="PSUM") as pst,
            tc.tile_pool(name="psf", bufs=2, space="PSUM") as psf,
            tc.tile_pool(name="psy", bufs=1, space="PSUM") as psyp,
        ):
            ident = const.tile([128, 128], BF16)
            make_identity(nc, ident)
            onesb_t = const.tile([1, 128], BF16)
            nc.sync.dma_start(out=onesb_t, in_=onesb_d[:, :])
            bf1_sb = const.tile([128, FB], F32)
            nc.sync.dma_start(out=bf1_sb,
                              in_=bf1_d[:, :].rearrange("f p -> p f"))
            bf2_sb = const.tile([1, D], BF16)
            nc.sync.dma_start(out=bf2_sb, in_=bf2_d[:, :])
            g2_sb = const.tile([128, D], F32)
            nc.sync.dma_start(out=g2_sb, in_=bass.AP(
                tensor=g2_d, offset=0, ap=[[0, 128], [1, D]]))
            b2_sb = const.tile([128, D], F32)
            nc.sync.dma_start(out=b2_sb, in_=bass.AP(
                tensor=b2_d, offset=0, ap=[[0, 128], [1, D]]))
            # residual 1 + LN1 + transpose
            lnT = lnTp.tile([128, CB, TOK_PC], BF16, name="lnT", tag="lnT")
            ao_t = []
            for nt in range(NT):
                at = xin.tile([128, D], BF16, name="at", tag="attn_in")
                nc.sync.dma_start(out=at,
                                  in_=attn_d[nt * 128:(nt + 1) * 128, :])
                xt = xin.tile([128, D], F32, name="xt", tag="x_in")
                nc.sync.dma_start(out=xt,
                                  in_=x_d[nt * 128:(nt + 1) * 128, :])
                den = small.tile([128, H], F32, name="den", tag="den")
                nc.sync.dma_start(out=den,
                                  in_=den_d[nt * 128:(nt + 1) * 128, :])
                rec = small.tile([128, H], F32, name="recd", tag="recd")
                nc.vector.reciprocal(out=rec, in_=den)
                an = lnp.tile([128, H, DH], F32, name="an", tag="an",
                              bufs=1)
                rec_bc = bass.AP(tensor=rec.tensor, offset=rec.offset,
                                 ap=[rec.ap[0], rec.ap[1], [0, DH]])
                nc.vector.tensor_tensor(
                    out=an, in0=at.rearrange("p (h d) -> p h d", h=H),
                    in1=rec_bc, op=OP.mult)
                ao = aop.tile([128, D], F32, name=f"ao{nt}", tag=f"ao{nt}")
                nc.gpsimd.tensor_tensor(
                    out=ao.rearrange("p (h d) -> p h d", h=H),
                    in0=an,
                    in1=xt.rearrange("p (h d) -> p h d", h=H), op=OP.add)
                ao_t.append(ao)
                mu, rstd = _ln_stats(nc, small, ao)
                ln_t = lnp.tile([128, D], BF16, name="ln3", tag="ln3")
                nc.vector.tensor_scalar(out=ln_t, in0=ao, scalar1=mu,
                                        scalar2=rstd, op0=OP.subtract,
                                        op1=OP.mult)
                for cb in range(CB):
                    tp = pst.tile([128, 128], BF16, name="tp", tag="tp")
                    nc.tensor.transpose(
                        tp, ln_t[:, cb * 128:(cb + 1) * 128], ident)
                    nc.vector.tensor_copy(
                        out=lnT[:, cb, nt * 128:(nt + 1) * 128], in_=tp)

            # second-half FFN2 weights issued now on the gpsimd queue:
            # after the ao adds, before the FFN1/FFN2 streams need the pipe
            w2r = w2rp.tile([128, FB, 512], BF16, name="w2r", tag="w2r")
            nc.gpsimd.dma_start(out=w2r, in_=wf2b_d[:, :, :])

            # FFN1: t1[f-part, fb, n], bf16 weights in 8 chunked DMAs,
            # bias+relu fused on ScalarE
            t1 = [t1p.tile([128, TOK_PC], BF16, name=f"t1_{fb}",
                           tag=f"t1_{fb}") for fb in range(FB)]
            for ch in range(8):
                w1 = w1p.tile([128, 4, CB, 128], BF16, name="w1", tag="w1")
                nc.sync.dma_start(out=w1, in_=wf1_d[ch])
                for f4 in range(4):
                    fb = ch * 4 + f4
                    ps = psf.tile([128, TOK_PC], F32, name="psf", tag="psf")
                    for cb in range(CB):
                        nc.tensor.matmul(ps, w1[:, f4, cb, :],
                                         lnT[:, cb, :],
                                         start=(cb == 0),
                                         stop=(cb == CB - 1))
                    nc.scalar.activation(out=t1[fb], in_=ps,
                                         func=AF.Relu,
                                         bias=bf1_sb[:, fb:fb + 1],
                                         scale=1.0)

            # FFN2 accumulating in psum; two passes over d halves
            y_t = [yp.tile([128, D], F32, name=f"y{nt}", tag=f"y{nt}")
                   for nt in range(NT)]
            # d-half 0: fb-outer with streamed weights (8 chunked DMAs)
            dsl0 = slice(0, 512)
            pys = [psyp.tile([128, 512], F32, name=f"psy{nt}",
                             tag=f"psy{nt}") for nt in range(NT)]
            for ch in range(8):
                w2 = w2p.tile([128, 4, 512], BF16, name="w2", tag="w2")
                nc.gpsimd.dma_start(out=w2,
                                    in_=wf2a_d[:, ch * 4:(ch + 1) * 4, :])
                for f4 in range(4):
                    fb = ch * 4 + f4
                    for nt in range(NT):
                        nc.tensor.matmul(
                            pys[nt], t1[fb][:, nt * 128:(nt + 1) * 128],
                            w2[:, f4, :], start=(fb == 0), stop=False)
            for nt in range(NT):
                nc.tensor.matmul(pys[nt], onesb_t, bf2_sb[0:1, dsl0],
                                 start=False, stop=True)
                nc.vector.tensor_tensor(out=y_t[nt][:, dsl0],
                                        in0=pys[nt],
                                        in1=ao_t[nt][:, dsl0],
                                        op=OP.add)
            # d-half 1: resident weights, nt-major so each token tile's
            # LN2 overlaps the remaining matmuls
            dsl1 = slice(512, 1024)
            for nt in range(NT):
                py = psyp.tile([128, 512], F32, name=f"psyb{nt}",
                               tag=f"psy{nt}")
                for fb in range(FB):
                    nc.tensor.matmul(
                        py, t1[fb][:, nt * 128:(nt + 1) * 128],
                        w2r[:, fb, :], start=(fb == 0), stop=False)
                nc.tensor.matmul(py, onesb_t, bf2_sb[0:1, dsl1],
                                 start=False, stop=True)
                nc.vector.tensor_tensor(out=y_t[nt][:, dsl1],
                                        in0=py,
                                        in1=ao_t[nt][:, dsl1],
                                        op=OP.add)
                mu, rstd = _ln_stats(nc, small, y_t[nt])
                z = lnp.tile([128, D], F32, name="z", tag="z")
                nc.vector.tensor_scalar(out=z, in0=y_t[nt], scalar1=mu,
                                        scalar2=rstd, op0=OP.subtract,
                                        op1=OP.mult)
                nc.vector.tensor_tensor(out=z, in0=z, in1=g2_sb,
                                        op=OP.mult)
                ot = outp.tile([128, D], F32, name="ot", tag="out")
                nc.vector.tensor_tensor(out=ot, in0=z, in1=b2_sb,
                                        op=OP.add)
                nc.scalar.dma_start(out=out_o[nt * 128:(nt + 1) * 128, :],
                                    in_=ot)

    nc.compile()
    return nc


_CACHE = {}


def _get(name, builder):
    if name not in _CACHE:
        _CACHE[name] = builder()
    return _CACHE[name]


def kernel(x, mask, Wq, bq, Wk, bk, Wv, bv, g_in, b_in, g1, b1,
           W_ff1, b_ff1, W_ff2, b_ff2, g2, b2):
    f = np.float32
    bf = ml_dtypes.bfloat16
    x = np.asarray(x, f)
    xf = x.reshape(NTOK, D)

    def wprep(W):
        Wf = np.asarray(g_in, f)[:, None] * np.asarray(W, f)
        return np.ascontiguousarray(
            Wf.reshape(CB, 128, D).transpose(1, 0, 2)).astype(bf)

    Wq_b, Wk_b, Wv_b = wprep(Wq), wprep(Wk), wprep(Wv)
    bq_f = (b_in @ Wq + bq).astype(f).reshape(CB, 128)
    bk_f = (b_in @ Wk + bk).astype(f).reshape(CB, 128)
    bv_f = np.asarray(b_in @ Wv + bv, f)
    Wf1_f = np.ascontiguousarray(
        (g1[:, None] * W_ff1).astype(f).reshape(CB, 128, 8, 4, 128)
        .transpose(2, 1, 3, 0, 4)).astype(bf)
    bf1_f = (b1 @ W_ff1 + b_ff1).astype(f).reshape(FB, 128)
    Wf2 = np.asarray(W_ff2, f)
    Wf2a = np.ascontiguousarray(
        Wf2[:, 0:512].reshape(FB, 128, 512).transpose(1, 0, 2)).astype(bf)
    Wf2b = np.ascontiguousarray(
        Wf2[:, 512:1024].reshape(FB, 128, 512).transpose(1, 0, 2)).astype(bf)
    bf2_f = np.asarray(b_ff2, bf).reshape(1, D)
    x3 = xf + bv_f[None, :]
    maskT = np.ascontiguousarray(
        np.asarray(mask)[:, 0].transpose(0, 2, 1)
        .reshape(B, MB, 128, T).transpose(0, 2, 1, 3)).astype(bf)

    cores = list(range(NCORES))

    # ---- phase 1
    nc1 = _get("p1", build_phase1)
    xb = xf.astype(bf)
    in1 = [{
        "x": xb[c * TOK_PC:(c + 1) * TOK_PC],
        "wq": Wq_b, "wk": Wk_b, "wv": Wv_b,
        "bq": bq_f, "bk": bk_f,
    } for c in cores]
    r1 = run_bass_kernel_spmd(nc1, in1, cores)
    qT = np.concatenate([r1.results[c]["qT"] for c in cores], axis=1)
    kT = np.concatenate([r1.results[c]["kT"] for c in cores], axis=1)
    v = np.concatenate([r1.results[c]["v"] for c in cores], axis=0)

    # ---- phase 2
    nc2 = _get("p2", build_phase2)
    onecol = np.ones((T, 1), bf)
    in2 = []
    for c in cores:
        qs, ks, vs = [], [], []
        for b in range(B):
            for hp in range(2):
                h = 2 * c + hp
                qs.append(qT[h * DH:(h + 1) * DH, b * T:(b + 1) * T])
                ks.append(kT[h * DH:(h + 1) * DH, b * T:(b + 1) * T])
                va = np.concatenate(
                    [v[b * T:(b + 1) * T, h * DH:(h + 1) * DH], onecol],
                    axis=1)
                vs.append(np.ascontiguousarray(
                    va.reshape(MB, 128, DH + 1).transpose(1, 0, 2)))
        in2.append({
            "qT": np.ascontiguousarray(np.stack(qs)),
            "kT": np.ascontiguousarray(np.stack(ks)),
            "v": np.ascontiguousarray(np.stack(vs)),
            "maskT": maskT,
        })
    r2 = run_bass_kernel_spmd(nc2, in2, cores)
    attn = np.empty((NTOK, D), bf)
    den = np.empty((NTOK, H), f)
    for c in cores:
        i = 0
        for b in range(B):
            for hp in range(2):
                h = 2 * c + hp
                a65 = r2.results[c]["attn"][i]
                attn[b * T:(b + 1) * T, h * DH:(h + 1) * DH] = a65[0:DH, :].T
                den[b * T:(b + 1) * T, h] = a65[DH, :].astype(f)
                i += 1

    # ---- phase 3
    nc3 = _get("p3", build_phase3)
    in3 = [{
        "attn": attn[c * TOK_PC:(c + 1) * TOK_PC],
        "den": den[c * TOK_PC:(c + 1) * TOK_PC],
        "x": x3[c * TOK_PC:(c + 1) * TOK_PC],
        "wf1": Wf1_f, "bf1": bf1_f, "wf2a": Wf2a, "wf2b": Wf2b,
        "bf2": bf2_f,
        "onesb": np.ones((1, 128), bf),
        "g2": np.asarray(g2, f).reshape(1, D),
        "b2": np.asarray(b2, f).reshape(1, D),
    } for c in cores]
    r3 = run_bass_kernel_spmd(nc3, in3, cores)
    out = np.concatenate([r3.results[c]["out"] for c in cores], axis=0)
    return out.reshape(B, T, D)
